# revision 24
# baseline (speedup 1.0000x reference)
"""Trainium2 Bass kernel for nn_DenseDSnetwork (DeepSets-over-subgraphs GNN readout).

Self-contained: kernel(**inputs) takes the FULL unsharded inputs, shards
subgraphs across 8 NeuronCores (whole graphs stay on one core; subgraph_idx
is sorted), runs a Bass/Tile kernel per core via run_bass_kernel_spmd, and
gathers the full [4096, 10] output.
"""
import sys
sys.path.insert(0, "/opt/trn_rl_repo")


import math
from contextlib import ExitStack

import numpy as np

import concourse.bass as bass
import concourse.bacc as bacc
import concourse.mybir as mybir
import concourse.tile as tile

BF16 = mybir.dt.bfloat16
F32 = mybir.dt.float32
AF = mybir.ActivationFunctionType
ALU = mybir.AluOpType

GB = 128          # graphs per block (= segsum matmul window = PSUM partitions)
GRP = 512         # rows per main-pass unit (= PSUM bank free size fp32)
GRPT = 16         # row-tiles per batched transpose/load slab


def make_cfg(S, G, D, L, H, T, ncores):
    assert D == 256 and H == 2 * D, "kernel is specialized to D=256"
    g_loc = G // ncores
    nblk = g_loc // GB
    assert g_loc % GB == 0
    return dict(S=S, G=G, D=D, L=L, H=H, T=T, ncores=ncores,
                g_loc=g_loc, nblk=nblk)


def host_prep(inputs, cfg):
    """Split/pad/transpose inputs into per-core in_maps. Returns (in_maps, meta)."""
    S, G, D, L, T = cfg["S"], cfg["G"], cfg["D"], cfg["L"], cfg["T"]
    ncores, g_loc, nblk = cfg["ncores"], cfg["g_loc"], cfg["nblk"]
    bf = np.dtype(mybir.dt.np(BF16))

    h = np.ascontiguousarray(np.asarray(inputs["h_subgraph"], np.float32))
    idx = np.asarray(inputs["subgraph_idx"]).astype(np.int64)
    assert h.shape == (S, D)
    assert np.all(np.diff(idx) >= 0), "subgraph_idx must be sorted"

    counts = np.bincount(idx, minlength=G).astype(np.float32)
    inv = (1.0 / np.maximum(counts, 1.0)).astype(np.float32)

    # block row ranges: block (c,b) covers graphs [g0, g0+GB)
    nblk_tot = ncores * nblk
    g_edges = np.arange(nblk_tot + 1) * GB
    r_edges = np.searchsorted(idx, g_edges)          # row boundaries
    blk_rows = np.diff(r_edges)
    RB = 128 * int(math.ceil(blk_rows.max() / 128.0))
    W = nblk * RB
    ntile = RB // 128

    fc_w = np.asarray(inputs["fc_w"], np.float32)
    fc_b = np.asarray(inputs["fc_b"], np.float32)
    fcs_w = np.asarray(inputs["fcs_w"], np.float32)
    fcs_b = np.asarray(inputs["fcs_b"], np.float32)
    f1_w = np.asarray(inputs["f1_w"], np.float32)
    f1_b = np.asarray(inputs["f1_b"], np.float32)
    f2_w = np.asarray(inputs["f2_w"], np.float32)
    f2_b = np.asarray(inputs["f2_b"], np.float32)

    # ---- packed bf16 weight/const array: one DMA ----
    # layout (cols): fcw (i,k,c)*128 | fcsw (i,k)*256 | f1w (k,m)*128 |
    #                f2w k*16 | ident 128 | iota 128
    O_FCW, O_FCSW, O_F1W, O_F2W = 0, 12 * 128, 12 * 128 + 6 * 256, 4096
    O_ID, O_IOTA = 4160, 4288
    WBF = 4416
    wbf = np.zeros((128, WBF), bf)
    for i in range(L):
        for k in range(2):
            for c in range(2):
                o = O_FCW + ((i * 2 + k) * 2 + c) * 128
                wbf[:, o:o + 128] = fc_w[i][128*k:128*k+128, 128*c:128*c+128].astype(bf)
            o = O_FCSW + (i * 2 + k) * 256
            wbf[:, o:o + 256] = fcs_w[i][128*k:128*k+128, :].astype(bf)
    for k in range(2):
        for m in range(4):
            o = O_F1W + (k * 4 + m) * 128
            wbf[:, o:o + 128] = f1_w[128*k:128*k+128, 128*m:128*m+128].astype(bf)
    for k in range(4):
        wbf[:, O_F2W + k*16: O_F2W + k*16 + T] = f2_w[128*k:128*k+128, :].astype(bf)
    wbf[:, O_ID:O_ID+128] = np.eye(128, dtype=bf)
    wbf[:, O_IOTA:O_IOTA+128] = np.tile(
        np.arange(128, dtype=np.float32).astype(bf)[None, :], (128, 1))

    # ---- packed f32 array (per-core lb/inv prepended per core below) ----
    # cols: lb nblk*ntile | inv nblk | bvecbc 3*256 | f1b 4 | f2b 1
    O_LB = 0
    O_INV = nblk * ntile
    O_BV = O_INV + nblk
    O_F1B = O_BV + L * 256
    O_F2B = O_F1B + 4
    WF32 = O_F2B + 1
    wf32_shared = np.zeros((128, WF32), np.float32)
    for i in range(L):
        bv = fc_b[i] + fcs_b[i]
        wf32_shared[:, O_BV + i*256: O_BV + (i+1)*256] = bv[None, :]
    for m in range(4):
        wf32_shared[:, O_F1B + m] = f1_b[128*m:128*m+128]
    wf32_shared[:T, O_F2B] = f2_b

    in_maps = []
    for c in range(ncores):
        hT = np.zeros((2, 128, W), bf)
        hrows = np.zeros((W, D), bf)
        ATd = np.zeros((nblk, 128, RB), bf)
        wf32 = wf32_shared.copy()
        lbd = np.full((128, nblk * ntile), 255.0, np.float32)
        for b in range(nblk):
            bi = c * nblk + b
            r0, r1 = int(r_edges[bi]), int(r_edges[bi + 1])
            n = r1 - r0
            rows = h[r0:r1].astype(bf).astype(np.float32)   # bf16-rounded
            for k in range(2):
                hT[k, :, b*RB:b*RB+n] = rows[:, 128*k:128*k+128].T.astype(bf)
            hrows[b*RB:b*RB+n, :] = rows.astype(bf)
            lb = (idx[r0:r1] - bi * GB).astype(np.int64)
            assert lb.min() >= 0 and lb.max() < GB
            j = np.arange(n)
            lbd[j % 128, b * ntile + j // 128] = lb.astype(np.float32)
            ATd[b][lb, j] = 1.0
            g0 = bi * GB
            wf32[:, O_INV + b] = inv[g0:g0+GB]
        wf32[:, O_LB:O_LB + nblk*ntile] = lbd
        in_maps.append(dict(hT=hT, hrows=hrows, ATd=ATd, wbf=wbf, wf32=wf32))
    meta = dict(RB=RB, W=W, r_edges=r_edges, WBF=WBF, WF32=WF32,
                offs=dict(O_FCW=O_FCW, O_FCSW=O_FCSW, O_F1W=O_F1W, O_F2W=O_F2W,
                          O_ID=O_ID, O_IOTA=O_IOTA, O_LB=O_LB, O_INV=O_INV,
                          O_BV=O_BV, O_F1B=O_F1B, O_F2B=O_F2B))
    return in_maps, meta


def build(cfg, meta, bench_loop=False):
    L, T = cfg["L"], cfg["T"]
    g_loc, nblk = cfg["g_loc"], cfg["nblk"]
    RB, W = meta["RB"], meta["W"]
    WBF, WF32 = meta["WBF"], meta["WF32"]
    OF = meta["offs"]
    ntile = RB // 128
    ngrp = (RB + GRP - 1) // GRP

    nc = bacc.Bacc("TRN2", target_bir_lowering=False, debug=False)

    hT_d = nc.dram_tensor("hT", [2, 128, W], BF16, kind="ExternalInput").ap()
    hrows_d = nc.dram_tensor("hrows", [W, 256], BF16, kind="ExternalInput").ap()
    AT_d = nc.dram_tensor("ATd", [nblk, 128, RB], BF16, kind="ExternalInput").ap()
    wbf_d = nc.dram_tensor("wbf", [128, WBF], BF16, kind="ExternalInput").ap()
    wf32_d = nc.dram_tensor("wf32", [128, WF32], F32, kind="ExternalInput").ap()
    out_d = nc.dram_tensor("outd", [T, g_loc], F32, kind="ExternalOutput").ap()
    niter_d = None
    if bench_loop:
        niter_d = nc.dram_tensor("niterd", [1, 1], mybir.dt.int32,
                                 kind="ExternalInput").ap()

    with tile.TileContext(nc) as tc, ExitStack() as ctx:
        hpool = ctx.enter_context(tc.tile_pool(name="h", bufs=1))
        wpool = ctx.enter_context(tc.tile_pool(name="w", bufs=1))
        hrpool = ctx.enter_context(tc.tile_pool(name="hr", bufs=4))
        mpool = ctx.enter_context(tc.tile_pool(name="m", bufs=2))
        tpool = ctx.enter_context(tc.tile_pool(name="t", bufs=1))
        x2pool = ctx.enter_context(tc.tile_pool(name="x2", bufs=4))
        epool = ctx.enter_context(tc.tile_pool(name="e", bufs=4))
        t2pool = ctx.enter_context(tc.tile_pool(name="t2", bufs=4))
        rpool = ctx.enter_context(tc.tile_pool(name="r", bufs=2))
        hidpool = ctx.enter_context(tc.tile_pool(name="hid", bufs=1))
        opool = ctx.enter_context(tc.tile_pool(name="o", bufs=1))
        # PSUM (8 banks): zps 4 | m_ps 2 | x2t 1 | mtx 1
        ps_z = ctx.enter_context(tc.tile_pool(name="psz", bufs=4, space="PSUM"))
        ps_m = ctx.enter_context(tc.tile_pool(name="psm", bufs=2, space="PSUM"))
        ps_x = ctx.enter_context(tc.tile_pool(name="psx", bufs=1, space="PSUM"))
        ps_t = ctx.enter_context(tc.tile_pool(name="pst", bufs=1, space="PSUM"))

        def load(dst, src):
            nc.scalar.dma_start(dst, src)

        # --- packed weights/consts: two DMAs (wbf = pure weights, hoistable;
        # wf32 = idx-derived lb/inv + biases, reloaded per iteration through a
        # 2-ring so the reload never serializes consecutive loop iterations) ---
        wbf_sb = wpool.tile([128, WBF], BF16, tag="wbf", name="wbf")
        load(wbf_sb[:], wbf_d[:])

        def fcw_ap(i, k, c):
            o = OF["O_FCW"] + ((i * 2 + k) * 2 + c) * 128
            return wbf_sb[:, o:o + 128]

        def fcsw_ap(i, k):
            o = OF["O_FCSW"] + (i * 2 + k) * 256
            return wbf_sb[:, o:o + 256]

        def f1w_ap(k, m):
            o = OF["O_F1W"] + (k * 4 + m) * 128
            return wbf_sb[:, o:o + 128]

        def f2w_ap(k):
            o = OF["O_F2W"] + k * 16
            return wbf_sb[:, o:o + T]

        ident_ap = wbf_sb[:, OF["O_ID"]:OF["O_ID"] + 128]
        iota_ap = wbf_sb[:, OF["O_IOTA"]:OF["O_IOTA"] + 128]

        wf32_holder = [None]

        def lb_ap(col):
            o = OF["O_LB"] + col
            return wf32_holder[0][:, o:o + 1]

        def inv_ap(b):
            o = OF["O_INV"] + b
            return wf32_holder[0][:, o:o + 1]

        def bv_ap(i):
            o = OF["O_BV"] + i * 256
            return wf32_holder[0][:, o:o + 256]

        def f1b_ap(m):
            o = OF["O_F1B"] + m
            return wf32_holder[0][:, o:o + 1]

        def f2b_ap():
            return wf32_holder[0][0:T, OF["O_F2B"]:OF["O_F2B"] + 1]

        at_sb = [wpool.tile([128, RB], BF16, tag=f"at{b}", name=f"at{b}")
                 for b in range(nblk)]
        a_full = [wpool.tile([128, RB], BF16, tag=f"ag{b}", name=f"ag{b}")
                  for b in range(nblk)]

        if bench_loop:
            from concourse.bass_types import RegisterHandles
            niter_sb = wpool.tile([1, 1], mybir.dt.int32, tag="niter", name="niter")
            nc.sync.dma_start(niter_sb[:], niter_d[:])
            _regs = []
            for _eng in (nc.sync, nc.scalar, nc.vector, nc.tensor, nc.gpsimd):
                _r = _eng.alloc_register(f"niter_{_eng.engine.name}")
                _eng.reg_load(_r, niter_sb[0:1, 0:1])
                _regs.append(_r)
            nval = nc.snap(RegisterHandles(_regs), min_val=1, max_val=100000)
            loop_cm = tc.For_i(0, nval, 1)
            loop_cm.__enter__()

        # --- per-iteration staging: idx-derived consts + h (in-place across
        # layers); hbuf(b)+AT(b) interleaved so block 0's operands land first
        wf32_sb = wpool.tile([128, WF32], F32, tag="wf32", bufs=2, name="wf32")
        wf32_holder[0] = wf32_sb
        load(wf32_sb[:], wf32_d[:])
        hbuf = {}
        for b in range(nblk):
            for k in range(2):
                hbuf[k, b] = hpool.tile([128, RB], BF16,
                                        tag=f"h{k}{b}", name=f"h{k}{b}")
                nc.sync.dma_start(hbuf[k, b][:], hT_d[k, :, b*RB:(b+1)*RB])
            nc.sync.dma_start(at_sb[b][:], AT_d[b])
        # a_full[b][p, t*128+g] = (lb[t*128+p] == g), built on the idle DVE
        for b in range(nblk):
            for t in range(ntile):
                nc.vector.tensor_scalar(
                    a_full[b][:, t*128:(t+1)*128], iota_ap,
                    lb_ap(b * ntile + t), None, ALU.is_equal)

        merge_cnt = [0]

        # ---- per-block helpers ----
        def seg_block(b, from_dram=False):
            """segment-sum of block b -> m_ps [128 g, 256 d] fp32 (psum)."""
            m_ps = ps_m.tile([128, 256], F32, tag="mps", name="mps")
            for t0 in range(0, ntile, GRPT):
                tn = min(GRPT, ntile - t0)
                hrbig = hrpool.tile([128, GRPT * 256], BF16, tag="hr", name="hr")
                if from_dram:
                    r0 = b * RB + t0 * 128
                    src = hrows_d[r0:r0 + tn*128, :].rearrange(
                        "(t p) d -> p t d", p=128)
                    load(hrbig[:, :tn*256].rearrange("p (t d) -> p t d", d=256), src)
                else:
                    for k in range(2):
                        oap = hrbig[:, :tn*256].rearrange(
                            "p (t k2 d) -> p t k2 d", k2=2, d=128)[:, :, k, :]
                        nc.sync.dma_start_transpose(
                            oap, hbuf[k, b][:, t0*128:(t0+tn)*128])
                for tt in range(tn):
                    t = t0 + tt
                    nc.tensor.matmul(m_ps[:],
                                     lhsT=a_full[b][:, t*128:(t+1)*128],
                                     rhs=hrbig[:, tt*256:(tt+1)*256],
                                     start=(t == 0), stop=(t == ntile - 1))
            return m_ps

        def mT_block(b, m_ps, mT_sb):
            """scale m_ps -> bf16, transpose into mT_sb[k][:, b*128:(b+1)*128]."""
            msb = mpool.tile([128, 256], BF16, tag="msb", name="msb")
            nc.scalar.activation(msb[:], m_ps[:], AF.Copy,
                                 scale=inv_ap(b))
            mtx = ps_t.tile([128, 256], BF16, tag="mtx", name="mtx")
            for k in range(2):
                nc.tensor.transpose(mtx[:, k*128:(k+1)*128],
                                    msb[:, 128*k:128*k+128], ident_ap)
            for k in range(2):
                nc.vector.tensor_copy(mT_sb[k][:, b*128:(b+1)*128],
                                      mtx[:, k*128:(k+1)*128])

        def x2_block(i, b, mT_sb):
            """x2 for block b in [128 g, 256 d] bf16 layout (bias folded)."""
            x2t = ps_x.tile([128, 256], F32, tag="x2t", name="x2t")
            for k in range(2):
                nc.tensor.matmul(x2t[:],
                                 lhsT=mT_sb[k][:, b*128:(b+1)*128],
                                 rhs=fcsw_ap(i, k),
                                 start=(k == 0), stop=(k == 1))
            x2sb = x2pool.tile([128, 256], BF16, tag="x2sb", name="x2sb")
            nc.vector.tensor_tensor(x2sb[:], x2t[:], bv_ap(i), ALU.add)
            return x2sb

        def main_block(i, b, x2sb):
            for j in range(ngrp):
                c0 = j * GRP
                n = min(GRP, RB - c0)
                zc = []
                for c in range(2):
                    zps = ps_z.tile([128, GRP], F32, tag="zps", name="zps")
                    for k in range(2):
                        nc.tensor.matmul(zps[:, :n], lhsT=fcw_ap(i, k, c),
                                         rhs=hbuf[k, b][:, c0:c0+n],
                                         start=(k == 0), stop=False)
                    nc.tensor.matmul(zps[:, :n],
                                     lhsT=x2sb[:, 128*c:128*c+128],
                                     rhs=at_sb[b][:, c0:c0+n],
                                     start=False, stop=True)
                    zc.append(zps)
                for c in range(2):
                    e_sb = epool.tile([128, GRP], BF16, tag="esb", name="esb")
                    nc.scalar.activation(e_sb[:, :n], zc[c][:, :n], AF.Exp)
                    t_sb = t2pool.tile([128, GRP], BF16, tag="tsb", name="tsb")
                    nc.vector.tensor_scalar(t_sb[:, :n], e_sb[:, :n],
                                            -1.0, 0.0, ALU.add, ALU.min)
                    # ELU = max(z,0) + min(exp(z)-1, 0); ~1/4 of merges take
                    # the Act(relu)+DVE(add) path to unload DVE's PSUM reads
                    if merge_cnt[0] % 4 == 3:
                        r_sb = rpool.tile([128, GRP], BF16, tag="rsb", name="rsb")
                        nc.scalar.activation(r_sb[:, :n], zc[c][:, :n], AF.Relu)
                        nc.vector.tensor_add(hbuf[c, b][:, c0:c0+n],
                                             r_sb[:, :n], t_sb[:, :n])
                    else:
                        nc.vector.scalar_tensor_tensor(
                            hbuf[c, b][:, c0:c0+n],
                            zc[c][:, :n], 0.0, t_sb[:, :n], ALU.max, ALU.add)
                    merge_cnt[0] += 1

        # --- layer 0: per-block interleave (seg reads DRAM rows, so mains can
        # start while later blocks' rows are still in flight on DMA) ---
        mT_sb = [tpool.tile([128, g_loc], BF16, tag=f"mT{k}", bufs=2,
                            name=f"mT0{k}") for k in range(2)]
        for b in range(nblk):
            m_ps = seg_block(b, from_dram=True)
            mT_block(b, m_ps, mT_sb)
            main_block(0, b, x2_block(0, b, mT_sb))

        # --- layers 1..L-1: phase-split (seg/mT/x2 for all blocks, then
        # mains) so cross-engine chains hide under PE matmuls; the head's
        # seg pass rides the last layer's main phase per block ---
        for i in range(1, L):
            mT_sb = [tpool.tile([128, g_loc], BF16, tag=f"mT{k}", bufs=2,
                                name=f"mT{i}{k}") for k in range(2)]
            x2s = []
            for b in range(nblk):
                m_ps = seg_block(b)
                mT_block(b, m_ps, mT_sb)
                x2s.append(x2_block(i, b, mT_sb))
            if i < L - 1:
                for b in range(nblk):
                    main_block(i, b, x2s[b])
            else:
                # last layer: head's seg pass rides each block's main
                mTh_sb = [tpool.tile([128, g_loc], BF16, tag=f"mT{k}", bufs=2,
                                     name=f"mTh{k}") for k in range(2)]
                for b in range(nblk):
                    main_block(i, b, x2s[b])
                    mT_block(b, seg_block(b), mTh_sb)

        # --- head f1/f2 ---
        hid_sb = []
        for m in range(4):
            hid_ps = ps_z.tile([128, g_loc], F32, tag="zps", name=f"hidps{m}")
            for k in range(2):
                nc.tensor.matmul(hid_ps[:],
                                 lhsT=f1w_ap(k, m), rhs=mTh_sb[k][:],
                                 start=(k == 0), stop=(k == 1))
            hs = hidpool.tile([128, g_loc], BF16, tag=f"hid{m}", name=f"hid{m}")
            nc.scalar.activation(hs[:], hid_ps[:], AF.Relu, bias=f1b_ap(m))
            hid_sb.append(hs)
        out_ps = ps_z.tile([128, g_loc], F32, tag="zps", name="outps")
        for k in range(4):
            nc.tensor.matmul(out_ps[0:T, :], lhsT=f2w_ap(k),
                             rhs=hid_sb[k][:], start=(k == 0), stop=(k == 3))
        out_sb = opool.tile([128, g_loc], F32, tag="outsb", name="outsb")
        nc.vector.tensor_scalar_add(out_sb[0:T, :], out_ps[0:T, :], f2b_ap())
        load(out_d[:, :], out_sb[0:T, :])
        if bench_loop:
            loop_cm.__exit__(None, None, None)

    nc.finalize()
    return nc


def unshard(results, cfg):
    """per-core outd [T, g_loc] -> full [G, T] fp32."""
    outs = [np.asarray(r["outd"]).T for r in results]   # [g_loc, T] each
    return np.concatenate(outs, axis=0).astype(np.float32)


_NCORES = 8


def kernel(**inputs):
    h = np.asarray(inputs["h_subgraph"])
    S, D = h.shape
    cfg = make_cfg(S=S, G=4096, D=D, L=3, H=2 * D, T=10, ncores=_NCORES)
    in_maps, meta = host_prep(inputs, cfg)
    nc = build(cfg, meta, bench_loop=False)
    from concourse import bass_utils
    res = bass_utils.run_bass_kernel_spmd(nc, in_maps, core_ids=list(range(_NCORES)))
    return unshard(res.results, cfg)
